# revision 1
# baseline (speedup 1.0000x reference)
"""Trainium2 Bass kernel for a 3-layer binary-weight MLP.

Problem (nn_MLP_56779467653689):
    x: [8192, 1024] f32
    h = relu(s0 * (x @ W0)) * 2      W0 = 2*k0-1  in {-1,+1}, [1024, 4096]
    h = relu(s1 * (h @ W1)) * 2      W1 [4096, 4096]
    out = s2 * (h @ W2)              W2 [4096, 1024]

Strategy: pure data-parallel over tokens across 8 NeuronCores (1024
tokens/core). Per core, activations live in SBUF as [features, tokens]
(features on partitions) so layers chain with no transposes. Weights are
pre-packed on the host into per-output-strip SBUF layout and streamed from
HBM. Matmuls run in bf16 (±1 weights are exact in bf16) with fp32 PSUM
accumulation; relu(s*acc)*2 == relu((2s)*acc) folds into one ACT per tile.
"""

from contextlib import ExitStack

import ml_dtypes
import numpy as np

P = 128
TOKENS = 8192
D_IN = 1024
D_H = 4096
D_OUT = 1024
N_CORES = 8
TOK_PER_CORE = TOKENS // N_CORES  # 1024
TOK_TILE = 512
NT = TOK_PER_CORE // TOK_TILE  # 2

BF16 = ml_dtypes.bfloat16

# Set TRACE=True (from test.py) to profile; LAST_EXEC_TIME_NS then holds the
# max per-core HW exec time of the most recent kernel() call.
TRACE = False
TRACE_CORES = None  # e.g. list(range(8)) to profile every core
LAST_EXEC_TIME_NS = None
LAST_RESULT = None

_cache = {}


def _dense_layer(nc, wpool, pspool, in_slice, w_dram, k_sub, n_t, evict,
                 t_outer=False):
    """out[n] strips = act(W[:, n-strip].T @ in) for n in range(n_t).

    in_slice(j, t): AP of the input block [P, TOK_TILE] for contraction
    tile j, token tile t. w_dram: packed [n_t, P, k_sub*P]. evict(n, t, ps)
    consumes the accumulated PSUM tile for (output strip n, token tile t).
    t_outer: each accumulation chain touches one token half, so layer 1's
    first chain starts after only the t=0 input halves landed.
    """
    import concourse.mybir as mybir

    for n in range(n_t):
        w = wpool.tile([P, k_sub * P], mybir.dt.bfloat16, tag="w", name=f"w_{n}")
        nc.sync.dma_start(out=w[:], in_=w_dram[n])
        if t_outer:
            for t in range(NT):
                ps = pspool.tile(
                    [P, TOK_TILE], mybir.dt.float32, tag="ps", name=f"ps_{n}_{t}"
                )
                for j in range(k_sub):
                    nc.tensor.matmul(
                        ps[:],
                        w[:, j * P : (j + 1) * P],
                        in_slice(j, t),
                        start=(j == 0),
                        stop=(j == k_sub - 1),
                    )
                evict(n, t, ps)
        else:
            # t-inner: consecutive matmuls alternate PSUM banks, which
            # measures ~0.7 ns/MM faster than same-bank accumulation runs.
            pss = [
                pspool.tile(
                    [P, TOK_TILE], mybir.dt.float32, tag="ps", name=f"ps_{n}_{t}"
                )
                for t in range(NT)
            ]
            for j in range(k_sub):
                for t in range(NT):
                    nc.tensor.matmul(
                        pss[t][:],
                        w[:, j * P : (j + 1) * P],
                        in_slice(j, t),
                        start=(j == 0),
                        stop=(j == k_sub - 1),
                    )
            for t in range(NT):
                evict(n, t, pss[t])


def _prune_dma_waits(nc, max_waits=1):
    """Drop transitively-implied waits from DMA instructions.

    DMA queue-entry descriptors hold a single sync wait; Tile's sem
    assignment is per-proc minimal but not transitively minimal across
    procs, so a recycled SBUF slot's DMA can carry WAR (engine) + WAW
    (prev slot writer's DMA lane) + lane-recycle waits = 3. The WAW (and
    often the recycle) wait is implied by the engine wait: the readers
    counted by the WAR threshold themselves waited on those DMAs.

    Soundness: a wait (s >= v) on instruction I is dropped only when the
    completion clocks implied by I's *other* waits already guarantee
    cumulative increments of s reached v. Completion clocks are built
    forward over the scheduled BIR order giving same-stream predecessor
    credit only to in-order engines (PE/ACT/DVE/SP), never to DMA lanes
    or Pool. Unrecognized wait/update modes contribute no credit, so
    unknowns can only inhibit pruning, never enable it.
    """
    import bisect

    import bass_rust

    IN_ORDER_ENGINES = {
        "EngineType.PE",
        "EngineType.Activation",
        "EngineType.DVE",
        "EngineType.SP",
    }

    sem_hist = {}  # sem -> ([cumulative values], [clocks at completion])
    sem_cum = {}  # sem -> cumulative increments so far
    eng_clock = {}  # engine -> completion clock of last instruction
    poisoned = set()  # sems with non-monotonic updates: no credit

    def cc(sem, val):
        """Completion clock implied by observing sem >= val, or None."""
        if sem in poisoned:
            return None
        hist = sem_hist.get(sem)
        if not hist or hist[0][-1] < val:
            return None
        return hist[1][bisect.bisect_left(hist[0], val)]

    def merge(dst, src):
        for k, v in src.items():
            if dst.get(k, 0) < v:
                dst[k] = v

    pruned = 0
    for bb in nc.m.functions[0].blocks:
        for inst in bb.instructions:
            si = inst.sync_info
            waits = list(si.on_wait or []) if si is not None else []
            ups = list(si.on_update or []) if si is not None else []
            is_dma = type(inst).__name__ == "InstDMACopy"

            clock = {}
            if not is_dma:
                prev = eng_clock.get(str(inst.engine))
                if prev is not None and str(inst.engine) in IN_ORDER_ENGINES:
                    merge(clock, prev)
            for w in waits:
                if w.wait_mode == "sem-ge-imm" and w.wait_value is not None:
                    c = cc(w.ant_name, w.wait_value)
                    if c is not None:
                        merge(clock, c)

            # Per-encoding wait budgets: DMA queue entries hold 1 wait;
            # engine instructions hold 2. Drain/EventSemaphore/control flow
            # are lowered specially by walrus — leave them alone.
            tname = type(inst).__name__
            if is_dma:
                cap = max_waits
            elif tname in ("InstDrain", "InstEventSemaphore", "InstCall",
                           "InstUnconditionalBranch", "InstISA"):
                cap = None
            else:
                cap = 2

            if cap is not None and len(waits) > cap:
                kept = list(waits)
                changed = True
                while len(kept) > cap and changed:
                    changed = False
                    for w in list(kept):
                        if w.wait_mode != "sem-ge-imm" or w.wait_value is None:
                            continue
                        implied = {}
                        provable = True
                        for o in kept:
                            if o is w:
                                continue
                            if o.wait_mode != "sem-ge-imm" or o.wait_value is None:
                                provable = False
                                break
                            c = cc(o.ant_name, o.wait_value)
                            if c is None:
                                provable = False
                                break
                            merge(implied, c)
                        if provable and implied.get(w.ant_name, 0) >= w.wait_value:
                            kept.remove(w)
                            pruned += 1
                            changed = True
                            break
                # Anything still over budget is left for Bacc's
                # generate_event_semaphores pass to split legally.
                if len(kept) != len(waits):
                    inst.sync_info = bass_rust.SyncInfo(on_wait=kept, on_update=ups)

            own = {}
            for u in ups:
                if u.update_mode not in ("sem-inc", "sem-add-imm"):
                    poisoned.add(u.ant_name)
                    continue
                inc = 1 if u.update_mode == "sem-inc" else u.update_value
                if inc is None:
                    poisoned.add(u.ant_name)
                    continue
                sem = u.ant_name
                sem_cum[sem] = sem_cum.get(sem, 0) + inc
                own[sem] = sem_cum[sem]
            merge(clock, own)
            for sem, cum in own.items():
                vals, clocks = sem_hist.setdefault(sem, ([], []))
                vals.append(cum)
                clocks.append(clock)
            if not is_dma:
                eng_clock[str(inst.engine)] = clock
    return pruned


def _build(a0, a1, a2):
    """Build the SPMD single-core program (same NEFF on all 8 cores)."""
    import concourse.mybir as mybir
    import concourse.tile as tile
    from concourse import bacc

    # Bacc (not plain Bass): its finalize() runs the wait-legalization
    # passes (move_matmul_waits_to_ldweights, generate_event_semaphores)
    # that split multi-wait instructions to the 1-wait HW encoding.
    nc = bacc.Bacc(
        "TRN2",
        target_bir_lowering=False,
        debug=False,
        enable_asserts=False,
        num_devices=N_CORES,
    )
    bf = mybir.dt.bfloat16
    f32 = mybir.dt.float32

    xt = nc.dram_tensor("xt", [D_IN, TOK_PER_CORE], bf, kind="ExternalInput")
    w0p = nc.dram_tensor("w0p", [D_H // P, P, D_IN], bf, kind="ExternalInput")
    w1p = nc.dram_tensor("w1p", [D_H // P, P, D_H], bf, kind="ExternalInput")
    w2p = nc.dram_tensor("w2p", [D_OUT // P, P, D_H], bf, kind="ExternalInput")
    outt = nc.dram_tensor("outt", [D_OUT, TOK_PER_CORE], f32, kind="ExternalOutput")

    relu = mybir.ActivationFunctionType.Relu

    with tile.TileContext(nc) as tc, ExitStack() as ctx:
        xpool = ctx.enter_context(tc.tile_pool(name="xp", bufs=1))
        h1pool = ctx.enter_context(tc.tile_pool(name="h1p", bufs=1))
        h2pool = ctx.enter_context(tc.tile_pool(name="h2p", bufs=1))
        wpool = ctx.enter_context(tc.tile_pool(name="wp", bufs=4))
        opool = ctx.enter_context(tc.tile_pool(name="op", bufs=3))
        pspool = ctx.enter_context(tc.tile_pool(name="psp", bufs=8, space="PSUM"))

        # x as per-j half-tiles in consumption order (t=0 first): the first
        # accumulation chain starts after just x_0_0 (128 KB) + one weight
        # strip, with later tiles streaming in behind the compute.
        x_half = [[None] * NT for _ in range(D_IN // P)]
        for t in range(NT):
            for j in range(D_IN // P):
                h = xpool.tile([P, TOK_TILE], bf, tag=f"x{j}_{t}", name=f"x_{j}_{t}")
                # ACT HWDGE queue: runs in parallel with the weight stream
                # on the SP queue, shortening the DMA-bound startup.
                nc.scalar.dma_start(
                    out=h[:],
                    in_=xt[j * P : (j + 1) * P, t * TOK_TILE : (t + 1) * TOK_TILE],
                )
                x_half[j][t] = h

        def x_slice(j, t):
            return x_half[j][t][:]

        h1_tiles = [
            h1pool.tile([P, TOK_PER_CORE], bf, tag=f"h1_{n}", name=f"h1_{n}")
            for n in range(D_H // P)
        ]
        h2_tiles = [
            h2pool.tile([P, TOK_PER_CORE], bf, tag=f"h2_{n}", name=f"h2_{n}")
            for n in range(D_H // P)
        ]

        def evict_h(h_tiles, scale):
            def evict(n, t, ps):
                nc.scalar.activation(
                    h_tiles[n][:, t * TOK_TILE : (t + 1) * TOK_TILE],
                    ps[:],
                    relu,
                    scale=scale,
                )

            return evict

        def evict_out(n, t, ps):
            # Stream each token half out as soon as its eviction lands —
            # the final strip's DMA starts one eviction earlier. The two
            # halves evict on different engines (ACT / DVE) so the last
            # strip's evictions run in parallel instead of serializing on
            # ScalarE right before the final DMA.
            o = opool.tile([P, TOK_TILE], f32, tag="o", name=f"o_{n}_{t}")
            if t % 2 == 0:
                nc.scalar.mul(o[:], ps[:], a2)
            else:
                nc.vector.tensor_scalar_mul(o[:], ps[:], a2)
            nc.scalar.dma_start(
                out=outt[n * P : (n + 1) * P, t * TOK_TILE : (t + 1) * TOK_TILE],
                in_=o[:],
            )

        def h_slice(h_tiles):
            return lambda j, t: h_tiles[j][:, t * TOK_TILE : (t + 1) * TOK_TILE]

        _dense_layer(nc, wpool, pspool, x_slice, w0p,
                     D_IN // P, D_H // P, evict_h(h1_tiles, a0), t_outer=True)
        _dense_layer(nc, wpool, pspool, h_slice(h1_tiles), w1p,
                     D_H // P, D_H // P, evict_h(h2_tiles, a1))
        _dense_layer(nc, wpool, pspool, h_slice(h2_tiles), w2p,
                     D_H // P, D_OUT // P, evict_out)

    _prune_dma_waits(nc)
    nc.finalize()
    return nc


def _pack_w(k):
    """Bool [K, N] -> bf16 ±1 packed [N/P, P, K]: strip n, partition p,
    free j*P+c  <-  W[j*P+p, n*P+c] (partition = contraction for lhsT)."""
    K, N = k.shape
    w = np.where(k, np.float32(1.0), np.float32(-1.0)).astype(BF16)
    return np.ascontiguousarray(
        w.reshape(K // P, P, N // P, P).transpose(2, 1, 0, 3).reshape(N // P, P, K)
    )


def _enable_ntff_trace():
    """Best-effort plumbing for trace=True under axon in this image.

    The image's ``antenv`` lacks the ``axon_hooks`` shim that
    ``trn_agent_boot`` would normally register the NTFF profile hook
    into, and there is no artifact bucket — stub both.
    """
    import sys
    import types

    import concourse.bass_utils as bu

    bu.upload_artifacts = lambda tmpdir: tmpdir
    try:
        from antenv import axon_hooks
    except ImportError:
        import antenv

        axon_hooks = types.ModuleType("antenv.axon_hooks")
        _state = {"hook": None}
        axon_hooks.set_axon_ntff_profile_hook = lambda h: _state.__setitem__(
            "hook", h
        )
        axon_hooks.get_axon_ntff_profile_hook = lambda: _state["hook"]
        sys.modules["antenv.axon_hooks"] = axon_hooks
        antenv.axon_hooks = axon_hooks
    if axon_hooks.get_axon_ntff_profile_hook() is None:
        from trn_agent_boot.trn_boot import _ntff_profile_via_ctypes

        axon_hooks.set_axon_ntff_profile_hook(
            _ntff_profile_via_ctypes("/opt/axon/libaxon_pjrt.so")
        )


def kernel(x, k0, k1, k2, s0, s1, s2):
    global LAST_EXEC_TIME_NS, LAST_RESULT
    from concourse.bass_utils import run_bass_kernel_spmd

    if TRACE:
        _enable_ntff_trace()

    x = np.asarray(x)
    a0 = 2.0 * float(np.asarray(s0))
    a1 = 2.0 * float(np.asarray(s1))
    a2 = float(np.asarray(s2))

    key = (a0, a1, a2)
    if key not in _cache:
        _cache[key] = _build(a0, a1, a2)
    nc = _cache[key]

    w0p = _pack_w(np.asarray(k0))
    w1p = _pack_w(np.asarray(k1))
    w2p = _pack_w(np.asarray(k2))

    in_maps = []
    for i in range(N_CORES):
        xs = x[i * TOK_PER_CORE : (i + 1) * TOK_PER_CORE].astype(BF16)
        in_maps.append(
            {
                "xt": np.ascontiguousarray(xs.T),
                "w0p": w0p,
                "w1p": w1p,
                "w2p": w2p,
            }
        )

    res = run_bass_kernel_spmd(
        nc, in_maps, list(range(N_CORES)), trace=TRACE, trace_cores=TRACE_CORES
    )
    LAST_EXEC_TIME_NS = res.exec_time_ns
    LAST_RESULT = res
    out = np.concatenate(
        [res.results[i]["outt"].T for i in range(N_CORES)], axis=0
    )
    return np.ascontiguousarray(out)



# revision 4
# speedup vs baseline: 1.1048x; 1.1048x over previous
"""Trainium2 Bass kernel for a 3-layer binary-weight MLP.

Problem (nn_MLP_56779467653689):
    x: [8192, 1024] f32
    h = relu(s0 * (x @ W0)) * 2      W0 = 2*k0-1  in {-1,+1}, [1024, 4096]
    h = relu(s1 * (h @ W1)) * 2      W1 [4096, 4096]
    out = s2 * (h @ W2)              W2 [4096, 1024]

Strategy: data-parallel over tokens across 8 NeuronCores (1024/core),
activations [features, tokens] in SBUF. Layers 1 and 2 use one level of
Winograd-Strassen: C = A@B with A = Wt (weights, [out, in]) and B = H
([in, tok]) split 2x2; the 7 A-side combos have small-integer entries, so
they are precomputed on the host with the eviction scale folded in (exact
in bf16). The 4 B-side combos (T1..T4) and the 7 output combines (U-adds)
run on the otherwise-idle Vector engine; relu evictions on Scalar. PE
multiply count for L1/L2 drops to 7/8 of the naive kernel.

SBUF is tight: T2 overwrites B12's slots (B12 is dead after the T build),
T4 is produced into a small rotating pool just-in-time, and h2 reuses the
x slots (x is dead once layer 0 finishes).
"""

from contextlib import ExitStack

import ml_dtypes
import numpy as np

P = 128
TOKENS = 8192
D_IN = 1024
D_H = 4096
D_OUT = 1024
N_CORES = 8
TOK_PER_CORE = TOKENS // N_CORES  # 1024
TOK_TILE = 512
NT = TOK_PER_CORE // TOK_TILE  # 2

BF16 = ml_dtypes.bfloat16

TRACE = False
TRACE_CORES = None
LAST_EXEC_TIME_NS = None
LAST_RESULT = None

_cache = {}


def _prune_dma_waits(nc, max_waits=1):
    """Drop transitively-implied waits from DMA instructions.

    DMA queue-entry descriptors hold a single sync wait; Tile's sem
    assignment is per-proc minimal but not transitively minimal across
    procs, so a recycled SBUF slot's DMA can carry WAR (engine) + WAW
    (prev slot writer's DMA lane) + lane-recycle waits = 3. The WAW (and
    often the recycle) wait is implied by the engine wait: the readers
    counted by the WAR threshold themselves waited on those DMAs.

    Soundness: a wait (s >= v) on instruction I is dropped only when the
    completion clocks implied by I's *other* waits already guarantee
    cumulative increments of s reached v. Completion clocks are built
    forward over the scheduled BIR order giving same-stream predecessor
    credit only to in-order engines (PE/ACT/DVE/SP), never to DMA lanes
    or Pool. Unrecognized wait/update modes contribute no credit, so
    unknowns can only inhibit pruning, never enable it.
    """
    import bisect

    import bass_rust

    IN_ORDER_ENGINES = {
        "EngineType.PE",
        "EngineType.Activation",
        "EngineType.DVE",
        "EngineType.SP",
    }

    sem_hist = {}
    sem_cum = {}
    eng_clock = {}
    poisoned = set()

    def cc(sem, val):
        if sem in poisoned:
            return None
        hist = sem_hist.get(sem)
        if not hist or hist[0][-1] < val:
            return None
        return hist[1][bisect.bisect_left(hist[0], val)]

    def merge(dst, src):
        for k, v in src.items():
            if dst.get(k, 0) < v:
                dst[k] = v

    pruned = 0
    for bb in nc.m.functions[0].blocks:
        for inst in bb.instructions:
            si = inst.sync_info
            waits = list(si.on_wait or []) if si is not None else []
            ups = list(si.on_update or []) if si is not None else []
            is_dma = type(inst).__name__ == "InstDMACopy"

            clock = {}
            if not is_dma:
                prev = eng_clock.get(str(inst.engine))
                if prev is not None and str(inst.engine) in IN_ORDER_ENGINES:
                    merge(clock, prev)
            for w in waits:
                if w.wait_mode == "sem-ge-imm" and w.wait_value is not None:
                    c = cc(w.ant_name, w.wait_value)
                    if c is not None:
                        merge(clock, c)

            tname = type(inst).__name__
            if is_dma:
                cap = max_waits
            elif tname in ("InstDrain", "InstEventSemaphore", "InstCall",
                           "InstUnconditionalBranch", "InstISA"):
                cap = None
            else:
                cap = 2

            if cap is not None and len(waits) > cap:
                kept = list(waits)
                changed = True
                while len(kept) > cap and changed:
                    changed = False
                    for w in list(kept):
                        if w.wait_mode != "sem-ge-imm" or w.wait_value is None:
                            continue
                        implied = {}
                        provable = True
                        for o in kept:
                            if o is w:
                                continue
                            if o.wait_mode != "sem-ge-imm" or o.wait_value is None:
                                provable = False
                                break
                            c = cc(o.ant_name, o.wait_value)
                            if c is None:
                                provable = False
                                break
                            merge(implied, c)
                        if provable and implied.get(w.ant_name, 0) >= w.wait_value:
                            kept.remove(w)
                            pruned += 1
                            changed = True
                            break
                if len(kept) != len(waits):
                    inst.sync_info = bass_rust.SyncInfo(on_wait=kept, on_update=ups)

            own = {}
            for u in ups:
                if u.update_mode not in ("sem-inc", "sem-add-imm"):
                    poisoned.add(u.ant_name)
                    continue
                inc = 1 if u.update_mode == "sem-inc" else u.update_value
                if inc is None:
                    poisoned.add(u.ant_name)
                    continue
                sem = u.ant_name
                sem_cum[sem] = sem_cum.get(sem, 0) + inc
                own[sem] = sem_cum[sem]
            merge(clock, own)
            for sem, cum in own.items():
                vals, clocks = sem_hist.setdefault(sem, ([], []))
                vals.append(cum)
                clocks.append(clock)
            if not is_dma:
                eng_clock[str(inst.engine)] = clock
    return pruned


def _build(a0):
    """SPMD single-core program. Layer-1/2 eviction scales are folded into
    the host-packed Strassen weight combos; only layer 0's scale (a0) is a
    runtime constant here."""
    import concourse.mybir as mybir
    import concourse.tile as tile
    from concourse import bacc

    nc = bacc.Bacc(
        "TRN2",
        target_bir_lowering=False,
        debug=False,
        enable_asserts=False,
        num_devices=N_CORES,
    )
    bf = mybir.dt.bfloat16
    f32 = mybir.dt.float32

    xt = nc.dram_tensor("xt", [D_IN, TOK_PER_CORE], bf, kind="ExternalInput")
    w0p = nc.dram_tensor("w0p", [D_H // P, P, D_IN], bf, kind="ExternalInput")
    w1s = nc.dram_tensor("w1s", [7, 16, P, D_H // 2], bf, kind="ExternalInput")
    w2s = nc.dram_tensor("w2s", [7, 4, P, D_H // 2], bf, kind="ExternalInput")
    outt = nc.dram_tensor("outt", [D_OUT, TOK_PER_CORE], f32, kind="ExternalOutput")

    relu = mybir.ActivationFunctionType.Relu

    with tile.TileContext(nc) as tc, ExitStack() as ctx:
        flat = ctx.enter_context(tc.tile_pool(name="flat", bufs=1))
        wpool = ctx.enter_context(tc.tile_pool(name="wp", bufs=8))
        upool = ctx.enter_context(tc.tile_pool(name="up", bufs=1))
        mpool = ctx.enter_context(tc.tile_pool(name="mp", bufs=2))
        tpool = ctx.enter_context(tc.tile_pool(name="tp", bufs=8))
        spool = ctx.enter_context(tc.tile_pool(name="sp", bufs=4))
        opool = ctx.enter_context(tc.tile_pool(name="op", bufs=4))
        pspool = ctx.enter_context(tc.tile_pool(name="psp", bufs=8, space="PSUM"))

        def ftile(tag):
            return flat.tile([P, TOK_TILE], bf, tag=tag, name=tag)

        # ---- layer 0: naive, x streamed on two queues ----
        x_half = [[None] * NT for _ in range(D_IN // P)]
        for t in range(NT):
            for j in range(D_IN // P):
                h = ftile(f"x{j}_{t}")
                q = nc.scalar if j % 2 == 0 else nc.gpsimd
                q.dma_start(
                    out=h[:],
                    in_=xt[j * P : (j + 1) * P, t * TOK_TILE : (t + 1) * TOK_TILE],
                )
                x_half[j][t] = h

        # h1 tile tags: (s, t) for s in 0..31; (s<16, t=1) lives in "b12" tags
        def h1tag(s, t):
            return f"b12_{s}" if (s < 16 and t == 1) else f"h1_{s}_{t}"

        h1 = {}
        for n in range(D_H // P):
            w = wpool.tile([P, D_IN], bf, tag="w", name=f"w0_{n}")
            nc.sync.dma_start(out=w[:], in_=w0p[n])
            for t in range(NT):
                ps = pspool.tile([P, TOK_TILE], f32, tag="ps", name=f"ps0_{n}_{t}")
                for j in range(D_IN // P):
                    nc.tensor.matmul(
                        ps[:],
                        w[:, j * P : (j + 1) * P],
                        x_half[j][t][:],
                        start=(j == 0),
                        stop=(j == D_IN // P - 1),
                    )
                ht = ftile(h1tag(n, t))
                nc.scalar.activation(ht[:], ps[:], relu, scale=a0)
                h1[(n, t)] = ht

        # ---- h2 tag map: reuse x slots (dead after L0), then fresh ----
        h2_tags = [f"x{j}_{t}" for t in range(NT) for j in range(D_IN // P)]
        h2_tags += [f"h2_{k}" for k in range(48)]
        h2_order = []
        for g in range(16):
            h2_order += [(g, 0), (16 + g, 0), (g, 1), (16 + g, 1)]
        h2map = {st: h2_tags[i] for i, st in enumerate(h2_order)}

        def strassen(nks, ngrp, w_dram, bsel, emit, tbuild):
            """One Strassen level: for each output-strip group, run the 7
            Mi chains (each contracting nks K-tiles) and combine.

            bsel(i, j) -> AP of B-block i's j-th strip; emit(g, name, ap_or
            psum...) handled inline below instead; tbuild() issues the
            T-combo builds once."""
            tbuild()
            for g in range(ngrp):
                pss = {}

                def chain(i):
                    wA = wpool.tile([P, 1024], bf, tag="w", name=f"w_{g}_{i}a")
                    wB = wpool.tile([P, 1024], bf, tag="w", name=f"w_{g}_{i}b")
                    nc.sync.dma_start(out=wA[:], in_=w_dram[i, g, :, 0:1024])
                    nc.gpsimd.dma_start(out=wB[:], in_=w_dram[i, g, :, 1024:2048])
                    ps = pspool.tile([P, TOK_TILE], f32, tag="ps", name=f"ps_{g}_{i}")
                    for j in range(nks):
                        w = wA if j < 8 else wB
                        jj = j if j < 8 else j - 8
                        nc.tensor.matmul(
                            ps[:],
                            w[:, jj * P : (jj + 1) * P],
                            bsel(i, j),
                            start=(j == 0),
                            stop=(j == nks - 1),
                        )
                    pss[i] = ps
                    return ps

                # Mi chains: 0:B11 4:T1 6:T3 1:B21 2:B22 5:T2 3:T4
                chain(0)
                m1s = mpool.tile([P, TOK_TILE], f32, tag="m1s", name=f"m1s_{g}")
                nc.scalar.copy(m1s[:], pss[0][:])
                chain(4)
                chain(6)
                chain(1)
                emit(g, "c11", m1s, pss[1], False)
                chain(2)
                chain(5)
                u2 = upool.tile([P, TOK_TILE], f32, tag="u2", name=f"u2_{g}")
                nc.vector.tensor_add(u2[:], m1s[:], pss[5][:])
                u4 = upool.tile([P, TOK_TILE], f32, tag="u4", name=f"u4_{g}")
                nc.vector.tensor_add(u4[:], u2[:], pss[4][:])
                u3 = upool.tile([P, TOK_TILE], f32, tag="u3", name=f"u3_{g}")
                nc.vector.tensor_add(u3[:], u2[:], pss[6][:])
                emit(g, "c22", u3, pss[4], False)
                emit(g, "c12", u4, pss[2], False)
                chain(3)
                emit(g, "c21", u3, pss[3], True)

        # ---- layer 1 strassen ----
        t1 = {}
        t3 = {}
        t2 = {}

        def tbuild1():
            for s in range(16):
                t1[s] = ftile(f"t1_{s}")
                nc.vector.tensor_sub(t1[s][:], h1[(s, 1)][:], h1[(s, 0)][:])
                t3[s] = ftile(f"t3_{s}")
                nc.vector.tensor_sub(t3[s][:], h1[(16 + s, 1)][:], h1[(s, 1)][:])
            for s in range(16):
                t2[s] = ftile(f"b12_{s}")  # overwrite B12 (dead after T1/T3)
                nc.vector.tensor_sub(t2[s][:], h1[(16 + s, 1)][:], t1[s][:])

        def bsel1(i, j):
            if i == 0:
                return h1[(j, 0)][:]
            if i == 1:
                return h1[(16 + j, 0)][:]
            if i == 2:
                return h1[(16 + j, 1)][:]
            if i == 4:
                return t1[j][:]
            if i == 5:
                return t2[j][:]
            if i == 6:
                return t3[j][:]
            # i == 3: T4 = T2 - B21, just-in-time
            t4 = tpool.tile([P, TOK_TILE], bf, tag="t4", name=f"t4_{j}")
            nc.vector.tensor_sub(t4[:], t2[j][:], h1[(16 + j, 0)][:])
            return t4[:]

        h2 = {}

        def emit1(g, cname, sb, ps, is_sub):
            st = {"c11": (g, 0), "c21": (16 + g, 0), "c12": (g, 1),
                  "c22": (16 + g, 1)}[cname]
            c = spool.tile([P, TOK_TILE], bf, tag="st", name=f"c_{g}_{cname}")
            if is_sub:
                nc.vector.tensor_sub(c[:], sb[:], ps[:])
            else:
                nc.vector.tensor_add(c[:], sb[:], ps[:])
            ht = ftile(h2map[st])
            nc.scalar.activation(ht[:], c[:], relu)
            h2[st] = ht

        strassen(16, 16, w1s, bsel1, emit1, tbuild1)

        # ---- layer 2 strassen (no relu; outputs DMA'd as they complete) ----
        t1b = {}
        t3b = {}
        t2b = {}

        def tbuild2():
            # T tiles reuse h1's slots (h1 dead after layer 1's chains)
            for s in range(16):
                t1b[s] = ftile(f"h1_{s}_0")
                nc.vector.tensor_sub(t1b[s][:], h2[(s, 1)][:], h2[(s, 0)][:])
                t3b[s] = ftile(f"h1_{16 + s}_0")
                nc.vector.tensor_sub(t3b[s][:], h2[(16 + s, 1)][:], h2[(s, 1)][:])
            for s in range(16):
                t2b[s] = ftile(f"h1_{16 + s}_1")
                nc.vector.tensor_sub(t2b[s][:], h2[(16 + s, 1)][:], t1b[s][:])

        def bsel2(i, j):
            if i == 0:
                return h2[(j, 0)][:]
            if i == 1:
                return h2[(16 + j, 0)][:]
            if i == 2:
                return h2[(16 + j, 1)][:]
            if i == 4:
                return t1b[j][:]
            if i == 5:
                return t2b[j][:]
            if i == 6:
                return t3b[j][:]
            t4 = tpool.tile([P, TOK_TILE], bf, tag="t4", name=f"t4b_{j}")
            nc.vector.tensor_sub(t4[:], t2b[j][:], h2[(16 + j, 0)][:])
            return t4[:]

        def emit2(g, cname, sb, ps, is_sub):
            row, t = {"c11": (g, 0), "c21": (4 + g, 0), "c12": (g, 1),
                      "c22": (4 + g, 1)}[cname]
            o = opool.tile([P, TOK_TILE], f32, tag="o", name=f"o_{g}_{cname}")
            if is_sub:
                nc.vector.tensor_sub(o[:], sb[:], ps[:])
            else:
                nc.vector.tensor_add(o[:], sb[:], ps[:])
            nc.scalar.dma_start(
                out=outt[row * P : (row + 1) * P,
                         t * TOK_TILE : (t + 1) * TOK_TILE],
                in_=o[:],
            )

        strassen(16, 4, w2s, bsel2, emit2, tbuild2)

    _prune_dma_waits(nc)
    nc.finalize()
    return nc


def _pack_w(k):
    """Bool [K, N] -> bf16 +-1 packed [N/P, P, K]."""
    K, N = k.shape
    w = np.where(k, np.float32(1.0), np.float32(-1.0)).astype(BF16)
    return np.ascontiguousarray(
        w.reshape(K // P, P, N // P, P).transpose(2, 1, 0, 3).reshape(N // P, P, K)
    )


def _pack_f(w):
    """Float [K, N] -> bf16 packed [N/P, P, K] (strip, partition=K, free)."""
    K, N = w.shape
    w = np.ascontiguousarray(w).astype(BF16)
    return np.ascontiguousarray(
        w.reshape(K // P, P, N // P, P).transpose(2, 1, 0, 3).reshape(N // P, P, K)
    )


def _strassen_weights(kmat, alpha):
    """Weight-side Winograd combos for C = Wt @ H, Wt = (2k-1).T scaled by
    alpha (folded eviction scale; exact in bf16 for power-of-two alpha).
    Returns [7, M/2/P, P, K/2] with Mi order [A11, A12, S4, A22, S1, S2, S3]
    matching B order [B11, B21, B22, T4, T1, T2, T3]."""
    Wt = np.where(kmat, 1.0, -1.0).astype(np.float32).T
    M, K = Wt.shape
    mh, kh = M // 2, K // 2
    A11, A12 = Wt[:mh, :kh], Wt[:mh, kh:]
    A21, A22 = Wt[mh:, :kh], Wt[mh:, kh:]
    S1 = A21 + A22
    S2 = S1 - A11
    S3 = A11 - A21
    S4 = A12 - S2
    packs = []
    for A in (A11, A12, S4, A22, S1, S2, S3):
        packs.append(_pack_f((alpha * A).T))
    return np.ascontiguousarray(np.stack(packs))


def _enable_ntff_trace():
    """Best-effort plumbing for trace=True under axon in this image."""
    import sys
    import types

    import concourse.bass_utils as bu

    bu.upload_artifacts = lambda tmpdir: tmpdir
    try:
        from antenv import axon_hooks
    except ImportError:
        import antenv

        axon_hooks = types.ModuleType("antenv.axon_hooks")
        _state = {"hook": None}
        axon_hooks.set_axon_ntff_profile_hook = lambda h: _state.__setitem__(
            "hook", h
        )
        axon_hooks.get_axon_ntff_profile_hook = lambda: _state["hook"]
        sys.modules["antenv.axon_hooks"] = axon_hooks
        antenv.axon_hooks = axon_hooks
    if axon_hooks.get_axon_ntff_profile_hook() is None:
        from trn_agent_boot.trn_boot import _ntff_profile_via_ctypes

        axon_hooks.set_axon_ntff_profile_hook(
            _ntff_profile_via_ctypes("/opt/axon/libaxon_pjrt.so")
        )


def kernel(x, k0, k1, k2, s0, s1, s2):
    global LAST_EXEC_TIME_NS, LAST_RESULT
    from concourse.bass_utils import run_bass_kernel_spmd

    if TRACE:
        _enable_ntff_trace()

    x = np.asarray(x)
    a0 = 2.0 * float(np.asarray(s0))
    a1 = 2.0 * float(np.asarray(s1))
    a2 = float(np.asarray(s2))

    key = (a0,)
    if key not in _cache:
        _cache[key] = _build(a0)
    nc = _cache[key]

    w0p = _pack_w(np.asarray(k0))
    w1s = _strassen_weights(np.asarray(k1), a1)
    w2s = _strassen_weights(np.asarray(k2), a2)

    in_maps = []
    for i in range(N_CORES):
        xs = x[i * TOK_PER_CORE : (i + 1) * TOK_PER_CORE].astype(BF16)
        in_maps.append(
            {
                "xt": np.ascontiguousarray(xs.T),
                "w0p": w0p,
                "w1s": w1s,
                "w2s": w2s,
            }
        )

    res = run_bass_kernel_spmd(
        nc, in_maps, list(range(N_CORES)), trace=TRACE, trace_cores=TRACE_CORES
    )
    LAST_EXEC_TIME_NS = res.exec_time_ns
    LAST_RESULT = res
    out = np.concatenate(
        [res.results[i]["outt"].T for i in range(N_CORES)], axis=0
    )
    return np.ascontiguousarray(out)


# revision 14
# speedup vs baseline: 1.1090x; 1.0038x over previous
"""Trainium2 Bass kernel for a 3-layer binary-weight MLP.

Problem (nn_MLP_56779467653689):
    x: [8192, 1024] f32
    h = relu(s0 * (x @ W0)) * 2      W0 = 2*k0-1  in {-1,+1}, [1024, 4096]
    h = relu(s1 * (h @ W1)) * 2      W1 [4096, 4096]
    out = s2 * (h @ W2)              W2 [4096, 1024]

Strategy: data-parallel over tokens across 8 NeuronCores (1024/core),
activations [features, tokens] in SBUF. Layers 1 and 2 use one level of
Winograd-Strassen: C = A@B with A = Wt (weights, [out, in]) and B = H
([in, tok]) split 2x2; the 7 A-side combos have small-integer entries, so
they are precomputed on the host with the eviction scale folded in (exact
in bf16). The 4 B-side combos (T1..T4) and the 7 output combines (U-adds)
run on the otherwise-idle Vector engine; relu evictions on Scalar. PE
multiply count for L1/L2 drops to 7/8 of the naive kernel.

SBUF is tight: T2 overwrites B12's slots (B12 is dead after the T build),
T4 is produced into a small rotating pool just-in-time, and h2 reuses the
x slots (x is dead once layer 0 finishes).
"""

from contextlib import ExitStack

import ml_dtypes
import numpy as np

P = 128
TOKENS = 8192
D_IN = 1024
D_H = 4096
D_OUT = 1024
N_CORES = 8
TOK_PER_CORE = TOKENS // N_CORES  # 1024
TOK_TILE = 512
NT = TOK_PER_CORE // TOK_TILE  # 2

BF16 = ml_dtypes.bfloat16

TRACE = False
TRACE_CORES = None
LAST_EXEC_TIME_NS = None
LAST_RESULT = None

_cache = {}


def _prune_dma_waits(nc, max_waits=1):
    """Drop transitively-implied waits from DMA instructions.

    DMA queue-entry descriptors hold a single sync wait; Tile's sem
    assignment is per-proc minimal but not transitively minimal across
    procs, so a recycled SBUF slot's DMA can carry WAR (engine) + WAW
    (prev slot writer's DMA lane) + lane-recycle waits = 3. The WAW (and
    often the recycle) wait is implied by the engine wait: the readers
    counted by the WAR threshold themselves waited on those DMAs.

    Soundness: a wait (s >= v) on instruction I is dropped only when the
    completion clocks implied by I's *other* waits already guarantee
    cumulative increments of s reached v. Completion clocks are built
    forward over the scheduled BIR order giving same-stream predecessor
    credit only to in-order engines (PE/ACT/DVE/SP), never to DMA lanes
    or Pool. Unrecognized wait/update modes contribute no credit, so
    unknowns can only inhibit pruning, never enable it.
    """
    import bisect

    import bass_rust

    IN_ORDER_ENGINES = {
        "EngineType.PE",
        "EngineType.Activation",
        "EngineType.DVE",
        "EngineType.SP",
    }

    sem_hist = {}
    sem_cum = {}
    eng_clock = {}
    poisoned = set()

    def cc(sem, val):
        if sem in poisoned:
            return None
        hist = sem_hist.get(sem)
        if not hist or hist[0][-1] < val:
            return None
        return hist[1][bisect.bisect_left(hist[0], val)]

    def merge(dst, src):
        for k, v in src.items():
            if dst.get(k, 0) < v:
                dst[k] = v

    pruned = 0
    for bb in nc.m.functions[0].blocks:
        for inst in bb.instructions:
            si = inst.sync_info
            waits = list(si.on_wait or []) if si is not None else []
            ups = list(si.on_update or []) if si is not None else []
            is_dma = type(inst).__name__ == "InstDMACopy"

            clock = {}
            if not is_dma:
                prev = eng_clock.get(str(inst.engine))
                if prev is not None and str(inst.engine) in IN_ORDER_ENGINES:
                    merge(clock, prev)
            for w in waits:
                if w.wait_mode == "sem-ge-imm" and w.wait_value is not None:
                    c = cc(w.ant_name, w.wait_value)
                    if c is not None:
                        merge(clock, c)

            tname = type(inst).__name__
            if is_dma:
                cap = max_waits
            elif tname in ("InstDrain", "InstEventSemaphore", "InstCall",
                           "InstUnconditionalBranch", "InstISA"):
                cap = None
            else:
                cap = 2

            if cap is not None and len(waits) > cap:
                kept = list(waits)
                changed = True
                while len(kept) > cap and changed:
                    changed = False
                    for w in list(kept):
                        if w.wait_mode != "sem-ge-imm" or w.wait_value is None:
                            continue
                        implied = {}
                        provable = True
                        for o in kept:
                            if o is w:
                                continue
                            if o.wait_mode != "sem-ge-imm" or o.wait_value is None:
                                provable = False
                                break
                            c = cc(o.ant_name, o.wait_value)
                            if c is None:
                                provable = False
                                break
                            merge(implied, c)
                        if provable and implied.get(w.ant_name, 0) >= w.wait_value:
                            kept.remove(w)
                            pruned += 1
                            changed = True
                            break
                if len(kept) != len(waits):
                    inst.sync_info = bass_rust.SyncInfo(on_wait=kept, on_update=ups)

            own = {}
            for u in ups:
                if u.update_mode not in ("sem-inc", "sem-add-imm"):
                    poisoned.add(u.ant_name)
                    continue
                inc = 1 if u.update_mode == "sem-inc" else u.update_value
                if inc is None:
                    poisoned.add(u.ant_name)
                    continue
                sem = u.ant_name
                sem_cum[sem] = sem_cum.get(sem, 0) + inc
                own[sem] = sem_cum[sem]
            merge(clock, own)
            for sem, cum in own.items():
                vals, clocks = sem_hist.setdefault(sem, ([], []))
                vals.append(cum)
                clocks.append(clock)
            if not is_dma:
                eng_clock[str(inst.engine)] = clock
    return pruned


def _build(a0):
    """SPMD single-core program. Layer-1/2 eviction scales are folded into
    the host-packed Strassen weight combos; only layer 0's scale (a0) is a
    runtime constant here."""
    import concourse.mybir as mybir
    import concourse.tile as tile
    from concourse import bacc

    nc = bacc.Bacc(
        "TRN2",
        target_bir_lowering=False,
        debug=False,
        enable_asserts=False,
        num_devices=N_CORES,
    )
    bf = mybir.dt.bfloat16
    f32 = mybir.dt.float32

    xt = nc.dram_tensor("xt", [NT, P, D_IN // P * TOK_TILE], bf,
                        kind="ExternalInput")
    w0p = nc.dram_tensor("w0p", [D_H // P, P, D_IN], bf, kind="ExternalInput")
    w1s = nc.dram_tensor("w1s", [7, 16, P, D_H // 2], bf, kind="ExternalInput")
    w2s = nc.dram_tensor("w2s", [7, 4, P, D_H // 2], bf, kind="ExternalInput")
    outt = nc.dram_tensor("outt", [D_OUT, TOK_PER_CORE], f32, kind="ExternalOutput")

    relu = mybir.ActivationFunctionType.Relu

    with tile.TileContext(nc) as tc, ExitStack() as ctx:
        flat = ctx.enter_context(tc.tile_pool(name="flat", bufs=1))
        wpool = ctx.enter_context(tc.tile_pool(name="wp", bufs=8))
        upool = ctx.enter_context(tc.tile_pool(name="up", bufs=1))
        mpool = ctx.enter_context(tc.tile_pool(name="mp", bufs=2))
        tpool = ctx.enter_context(tc.tile_pool(name="tp", bufs=8))
        spool = ctx.enter_context(tc.tile_pool(name="sp", bufs=4))
        opool = ctx.enter_context(tc.tile_pool(name="op", bufs=4))
        pspool = ctx.enter_context(tc.tile_pool(name="psp", bufs=8, space="PSUM"))

        def ftile(tag):
            return flat.tile([P, TOK_TILE], bf, tag=tag, name=tag)

        # ---- layer 0: naive. x is host-packed to [t, 128, j*512+c] so each
        # half loads as two wide DMAs (4KB row chunks) split across queues.
        xbig = []
        for t in range(NT):
            xb = flat.tile([P, D_IN // P * TOK_TILE], bf, tag=f"xb_{t}",
                           name=f"xb_{t}")
            half = D_IN // P * TOK_TILE // 2
            nc.scalar.dma_start(out=xb[:, 0:half], in_=xt[t, :, 0:half])
            nc.gpsimd.dma_start(out=xb[:, half:], in_=xt[t, :, half:])
            xbig.append(xb)

        def x_sl(j, t):
            return xbig[t][:, j * TOK_TILE : (j + 1) * TOK_TILE]

        # h1 tile tags: (s, t) for s in 0..31; (s<16, t=1) lives in "b12" tags
        def h1tag(s, t):
            return f"b12_{s}" if (s < 16 and t == 1) else f"h1_{s}_{t}"

        h1 = {}
        for n in range(D_H // P):
            w = wpool.tile([P, D_IN], bf, tag="w", name=f"w0_{n}")
            nc.sync.dma_start(out=w[:], in_=w0p[n])
            for t in range(NT):
                ps = pspool.tile([P, TOK_TILE], f32, tag="ps", name=f"ps0_{n}_{t}")
                for j in range(D_IN // P):
                    nc.tensor.matmul(
                        ps[:],
                        w[:, j * P : (j + 1) * P],
                        x_sl(j, t),
                        start=(j == 0),
                        stop=(j == D_IN // P - 1),
                    )
                ht = ftile(h1tag(n, t))
                nc.scalar.activation(ht[:], ps[:], relu, scale=a0)
                h1[(n, t)] = ht

        # ---- h2 slot map: first 16 tiles reuse the xb region (x is dead
        # once layer 0 finishes, before any h2 write) ----
        h2_order = []
        for g in range(16):
            h2_order += [(g, 0), (16 + g, 0), (g, 1), (16 + g, 1)]
        h2map = {}
        for i, st in enumerate(h2_order):
            if i < 16:
                h2map[st] = xbig[i // 8][:, (i % 8) * TOK_TILE :
                                         (i % 8 + 1) * TOK_TILE]
            else:
                h2map[st] = ftile(f"h2_{i - 16}")[:]

        def strassen(nks, ngrp, w_dram, bsel, emit, tbuild):
            """One Strassen level: for each output-strip group, run the 7
            Mi chains (each contracting nks K-tiles) and combine.

            bsel(i, j) -> AP of B-block i's j-th strip; emit(g, name, ap_or
            psum...) handled inline below instead; tbuild() issues the
            T-combo builds once."""
            tbuild()
            for g in range(ngrp):
                pss = {}

                def chain(*idxs):
                    # interleaved chains alternate PSUM banks between MMs
                    tiles = []
                    for i in idxs:
                        wA = wpool.tile([P, 1024], bf, tag="w", name=f"w_{g}_{i}a")
                        wB = wpool.tile([P, 1024], bf, tag="w", name=f"w_{g}_{i}b")
                        nc.sync.dma_start(out=wA[:], in_=w_dram[i, g, :, 0:1024])
                        nc.gpsimd.dma_start(
                            out=wB[:], in_=w_dram[i, g, :, 1024:2048]
                        )
                        ps = pspool.tile(
                            [P, TOK_TILE], f32, tag="ps", name=f"ps_{g}_{i}"
                        )
                        pss[i] = ps
                        tiles.append((i, wA, wB, ps))
                    for j in range(nks):
                        for i, wA, wB, ps in tiles:
                            w = wA if j < 8 else wB
                            jj = j if j < 8 else j - 8
                            nc.tensor.matmul(
                                ps[:],
                                w[:, jj * P : (jj + 1) * P],
                                bsel(i, j),
                                start=(j == 0),
                                stop=(j == nks - 1),
                            )

                # Mi chains: 0:B11 4:T1 6:T3 1:B21 2:B22 5:T2 3:T4
                chain(0, 4)
                m1s = mpool.tile([P, TOK_TILE], f32, tag="m1s", name=f"m1s_{g}")
                nc.scalar.copy(m1s[:], pss[0][:])
                chain(6, 1)
                emit(g, "c11", m1s, pss[1], False)
                chain(2, 5)
                u2 = upool.tile([P, TOK_TILE], f32, tag="u2", name=f"u2_{g}")
                nc.vector.tensor_add(u2[:], m1s[:], pss[5][:])
                u4 = upool.tile([P, TOK_TILE], f32, tag="u4", name=f"u4_{g}")
                nc.vector.tensor_add(u4[:], u2[:], pss[4][:])
                u3 = upool.tile([P, TOK_TILE], f32, tag="u3", name=f"u3_{g}")
                nc.vector.tensor_add(u3[:], u2[:], pss[6][:])
                emit(g, "c22", u3, pss[4], False)
                emit(g, "c12", u4, pss[2], False)
                chain(3)
                emit(g, "c21", u3, pss[3], True)

        # ---- layer 1 strassen ----
        t1 = {}
        t3 = {}
        t2 = {}

        def tbuild1():
            for s in range(16):
                t1[s] = ftile(f"t1_{s}")
                nc.vector.tensor_sub(t1[s][:], h1[(s, 1)][:], h1[(s, 0)][:])
                t3[s] = ftile(f"t3_{s}")
                nc.vector.tensor_sub(t3[s][:], h1[(16 + s, 1)][:], h1[(s, 1)][:])
            for s in range(16):
                t2[s] = ftile(f"b12_{s}")  # overwrite B12 (dead after T1/T3)
                nc.vector.tensor_sub(t2[s][:], h1[(16 + s, 1)][:], t1[s][:])

        def bsel1(i, j):
            if i == 0:
                return h1[(j, 0)][:]
            if i == 1:
                return h1[(16 + j, 0)][:]
            if i == 2:
                return h1[(16 + j, 1)][:]
            if i == 4:
                return t1[j][:]
            if i == 5:
                return t2[j][:]
            if i == 6:
                return t3[j][:]
            # i == 3: T4 = T2 - B21, just-in-time
            t4 = tpool.tile([P, TOK_TILE], bf, tag="t4", name=f"t4_{j}")
            nc.vector.tensor_sub(t4[:], t2[j][:], h1[(16 + j, 0)][:])
            return t4[:]

        h2 = {}

        def emit1(g, cname, sb, ps, is_sub):
            st = {"c11": (g, 0), "c21": (16 + g, 0), "c12": (g, 1),
                  "c22": (16 + g, 1)}[cname]
            c = spool.tile([P, TOK_TILE], bf, tag="st", name=f"c_{g}_{cname}")
            if is_sub:
                nc.vector.tensor_sub(c[:], sb[:], ps[:])
            else:
                nc.vector.tensor_add(c[:], sb[:], ps[:])
            ht = h2map[st]
            nc.scalar.activation(ht, c[:], relu)
            h2[st] = ht

        strassen(16, 16, w1s, bsel1, emit1, tbuild1)

        # ---- layer 2 strassen (no relu; outputs DMA'd as they complete) ----
        t1b = {}
        t3b = {}
        t2b = {}

        def tbuild2():
            # T tiles reuse h1's slots (h1 dead after layer 1's chains)
            for s in range(16):
                t1b[s] = ftile(f"h1_{s}_0")
                nc.vector.tensor_sub(t1b[s][:], h2[(s, 1)], h2[(s, 0)])
                t3b[s] = ftile(f"h1_{16 + s}_0")
                nc.vector.tensor_sub(t3b[s][:], h2[(16 + s, 1)], h2[(s, 1)])
            for s in range(16):
                t2b[s] = ftile(f"h1_{16 + s}_1")
                nc.vector.tensor_sub(t2b[s][:], h2[(16 + s, 1)], t1b[s][:])

        def bsel2(i, j):
            if i == 0:
                return h2[(j, 0)]
            if i == 1:
                return h2[(16 + j, 0)]
            if i == 2:
                return h2[(16 + j, 1)]
            if i == 4:
                return t1b[j][:]
            if i == 5:
                return t2b[j][:]
            if i == 6:
                return t3b[j][:]
            t4 = tpool.tile([P, TOK_TILE], bf, tag="t4", name=f"t4b_{j}")
            nc.vector.tensor_sub(t4[:], t2b[j][:], h2[(16 + j, 0)])
            return t4[:]

        def emit2(g, cname, sb, ps, is_sub):
            row, t = {"c11": (g, 0), "c21": (4 + g, 0), "c12": (g, 1),
                      "c22": (4 + g, 1)}[cname]
            o = opool.tile([P, TOK_TILE], f32, tag="o", name=f"o_{g}_{cname}")
            if is_sub:
                nc.vector.tensor_sub(o[:], sb[:], ps[:])
            else:
                nc.vector.tensor_add(o[:], sb[:], ps[:])
            q = nc.scalar if t == 0 else nc.gpsimd
            q.dma_start(
                out=outt[row * P : (row + 1) * P,
                         t * TOK_TILE : (t + 1) * TOK_TILE],
                in_=o[:],
            )

        strassen(16, 4, w2s, bsel2, emit2, tbuild2)

    _prune_dma_waits(nc)
    nc.finalize()
    return nc


def _pack_w(k):
    """Bool [K, N] -> bf16 +-1 packed [N/P, P, K]."""
    K, N = k.shape
    w = np.where(k, np.float32(1.0), np.float32(-1.0)).astype(BF16)
    return np.ascontiguousarray(
        w.reshape(K // P, P, N // P, P).transpose(2, 1, 0, 3).reshape(N // P, P, K)
    )


def _pack_f(w):
    """Float [K, N] -> bf16 packed [N/P, P, K] (strip, partition=K, free)."""
    K, N = w.shape
    w = np.ascontiguousarray(w).astype(BF16)
    return np.ascontiguousarray(
        w.reshape(K // P, P, N // P, P).transpose(2, 1, 0, 3).reshape(N // P, P, K)
    )


def _strassen_weights(kmat, alpha):
    """Weight-side Winograd combos for C = Wt @ H, Wt = (2k-1).T scaled by
    alpha (folded eviction scale; exact in bf16 for power-of-two alpha).
    Returns [7, M/2/P, P, K/2] with Mi order [A11, A12, S4, A22, S1, S2, S3]
    matching B order [B11, B21, B22, T4, T1, T2, T3]."""
    Wt = np.where(kmat, 1.0, -1.0).astype(np.float32).T
    M, K = Wt.shape
    mh, kh = M // 2, K // 2
    A11, A12 = Wt[:mh, :kh], Wt[:mh, kh:]
    A21, A22 = Wt[mh:, :kh], Wt[mh:, kh:]
    S1 = A21 + A22
    S2 = S1 - A11
    S3 = A11 - A21
    S4 = A12 - S2
    packs = []
    for A in (A11, A12, S4, A22, S1, S2, S3):
        packs.append(_pack_f((alpha * A).T))
    return np.ascontiguousarray(np.stack(packs))


def _enable_ntff_trace():
    """Best-effort plumbing for trace=True under axon in this image."""
    import sys
    import types

    import concourse.bass_utils as bu

    bu.upload_artifacts = lambda tmpdir: tmpdir
    try:
        from antenv import axon_hooks
    except ImportError:
        import antenv

        axon_hooks = types.ModuleType("antenv.axon_hooks")
        _state = {"hook": None}
        axon_hooks.set_axon_ntff_profile_hook = lambda h: _state.__setitem__(
            "hook", h
        )
        axon_hooks.get_axon_ntff_profile_hook = lambda: _state["hook"]
        sys.modules["antenv.axon_hooks"] = axon_hooks
        antenv.axon_hooks = axon_hooks
    if axon_hooks.get_axon_ntff_profile_hook() is None:
        from trn_agent_boot.trn_boot import _ntff_profile_via_ctypes

        axon_hooks.set_axon_ntff_profile_hook(
            _ntff_profile_via_ctypes("/opt/axon/libaxon_pjrt.so")
        )


def kernel(x, k0, k1, k2, s0, s1, s2):
    global LAST_EXEC_TIME_NS, LAST_RESULT
    from concourse.bass_utils import run_bass_kernel_spmd

    if TRACE:
        _enable_ntff_trace()

    x = np.asarray(x)
    a0 = 2.0 * float(np.asarray(s0))
    a1 = 2.0 * float(np.asarray(s1))
    a2 = float(np.asarray(s2))

    key = (a0,)
    if key not in _cache:
        _cache[key] = _build(a0)
    nc = _cache[key]

    w0p = _pack_w(np.asarray(k0))
    w1s = _strassen_weights(np.asarray(k1), a1)
    w2s = _strassen_weights(np.asarray(k2), a2)

    in_maps = []
    for i in range(N_CORES):
        xs = x[i * TOK_PER_CORE : (i + 1) * TOK_PER_CORE].astype(BF16)
        xsT = np.ascontiguousarray(xs.T)  # [feat, tok]
        xp = np.ascontiguousarray(
            xsT.reshape(D_IN // P, P, NT, TOK_TILE)
            .transpose(2, 1, 0, 3)
            .reshape(NT, P, D_IN // P * TOK_TILE)
        )
        in_maps.append(
            {
                "xt": xp,
                "w0p": w0p,
                "w1s": w1s,
                "w2s": w2s,
            }
        )

    res = run_bass_kernel_spmd(
        nc, in_maps, list(range(N_CORES)), trace=TRACE, trace_cores=TRACE_CORES
    )
    LAST_EXEC_TIME_NS = res.exec_time_ns
    LAST_RESULT = res
    out = np.concatenate(
        [res.results[i]["outt"].T for i in range(N_CORES)], axis=0
    )
    return np.ascontiguousarray(out)
